# revision 3
# baseline (speedup 1.0000x reference)
"""Trainium2 Bass kernel for nn_Attention_46110768890297.

Math: reference computes per-head q,k,v projections, causal-masked scores
with -inf, max-subtraction (NO softmax), then s @ v.  Because masked
positions carry -inf into the final matmul, output rows l < SEQ-1 are
exactly determined by IEEE rules from the SIGNS of v over the suffix
m > l:  all v>0 -> -inf, all v<0 -> +inf, else NaN.  Only row SEQ-1 is
finite and needs the actual attention computation.

Device computation per core (core = (head, batch-group of 5)):
  kT, vT = (h @ WkT + bk).T, (h @ WvT + bv).T   in [d, t] layout (t reversed)
  q999   = h[:, 999] @ WqT + bq
  s      = (q999 . kT) / sqrt(d); s -= max(s); out999 = sum_t s[t] * vT[:, t]
  rows 0..998: exclusive prefix min/max scans of vT along reversed t give
  the all-pos / all-neg suffix classification -> write -inf/+inf/NaN.
Host reassembles the 8 per-core [5, 1024, 1000] blocks.
"""

import numpy as np

import concourse.bass as bass
from concourse import mybir
from concourse.tile import TileContext
from concourse.bass_utils import run_bass_kernel_spmd

BATCH, SEQ, HIDDEN = 10, 1000, 4096
HEAD_DIM, NUM_HEAD = 1024, 4
NCORES = 8
BPC = BATCH // 2          # batches per core (2 batch groups)
C = HIDDEN // 128         # 32 contraction chunks
M = HEAD_DIM // 128       # 8 output-dim chunks
NT = 500                  # t-tile (PSUM free dim limit 512 fp32)
NH = SEQ // NT            # 2 t halves
F32 = mybir.dt.float32
U32 = mybir.dt.uint32
ALU = mybir.AluOpType
AF = mybir.ActivationFunctionType
INV_SQRT_D = 1.0 / float(np.sqrt(np.float32(HEAD_DIM)))


def split_multi_waits(nc):
    """This walrus build allows only ONE sync-wait per instruction struct.
    Move extra waits onto standalone EventSemaphore instructions placed
    immediately before, on the same engine (sequencers are in-order)."""
    k = 0
    for f in nc.m.functions:
        for bb in f.blocks:
            out, changed = [], False
            for inst in bb.instructions:
                si = inst.sync_info
                waits = list(si.on_wait) if (si is not None and si.on_wait) else []
                if len(waits) > 1:
                    for w in waits[:-1]:
                        ev = mybir.InstEventSemaphore(
                            name=f"xw-{k}", engine=inst.engine, debug=inst.debug,
                            sync_info=mybir.SyncInfo(on_wait=[w], on_update=[]))
                        k += 1
                        out.append(ev)
                    inst.sync_info = mybir.SyncInfo(
                        on_wait=[waits[-1]], on_update=list(si.on_update or []))
                    changed = True
                out.append(inst)
            if changed:
                bb.instructions = out
    return nc


def build_program():
    nc = bass.Bass()
    hTr = nc.declare_dram_parameter("hTr", [BPC, HIDDEN, SEQ], F32, isOutput=False)
    h999T = nc.declare_dram_parameter("h999T", [HIDDEN, BPC], F32, isOutput=False)
    WqT = nc.declare_dram_parameter("WqT", [HIDDEN, HEAD_DIM], F32, isOutput=False)
    WkT = nc.declare_dram_parameter("WkT", [HIDDEN, HEAD_DIM], F32, isOutput=False)
    WvT = nc.declare_dram_parameter("WvT", [HIDDEN, HEAD_DIM], F32, isOutput=False)
    bq = nc.declare_dram_parameter("bq", [HEAD_DIM], F32, isOutput=False)
    bk = nc.declare_dram_parameter("bk", [HEAD_DIM], F32, isOutput=False)
    bv = nc.declare_dram_parameter("bv", [HEAD_DIM], F32, isOutput=False)
    out = nc.declare_dram_parameter("out", [BPC, HEAD_DIM, SEQ], F32, isOutput=True)

    with TileContext(nc) as tc:
        with (
            tc.tile_pool(name="persist", bufs=1) as persist,
            tc.tile_pool(name="wstrip", bufs=3) as wpool,
            tc.tile_pool(name="hblk", bufs=C) as hpool,
            tc.tile_pool(name="proj", bufs=1) as projpool,
            tc.tile_pool(name="cls", bufs=2) as clspool,
            tc.tile_pool(name="ps", bufs=8, space="PSUM") as ps,
        ):
            # ---- constants / small loads ----
            h999_sb = persist.tile([128, C, BPC], F32)
            nc.sync.dma_start(out=h999_sb, in_=h999T.rearrange("(c p) b -> p c b", p=128))
            bq_sb = persist.tile([128, M], F32)
            nc.sync.dma_start(out=bq_sb, in_=bq.rearrange("(m p) -> p m", p=128))
            bk_sb = persist.tile([128, M], F32)
            nc.sync.dma_start(out=bk_sb, in_=bk.rearrange("(m p) -> p m", p=128))
            bv_sb = persist.tile([128, M], F32)
            nc.sync.dma_start(out=bv_sb, in_=bv.rearrange("(m p) -> p m", p=128))
            ones_sb = persist.tile([1, 128], F32)
            nc.vector.memset(ones_sb, 1.0)
            nan_c = persist.tile([128, 1], F32)
            nc.vector.memset(nan_c, float("nan"))
            ninf_c = persist.tile([128, 1], F32)
            nc.vector.memset(ninf_c, float("-inf"))
            pinf_c = persist.tile([128, 1], F32)
            nc.vector.memset(pinf_c, float("inf"))

            # ---- q999 for all 5 batches: q999_sb[:, m*BPC + b] ----
            # one PSUM tile per m-chunk: start=True clears has_written for the
            # whole bank, so interleaved groups must not share a bank
            psq = [ps.tile([128, BPC], F32, tag="pp", name=f"psq{m}") for m in range(M)]
            for c in range(C):
                ws = wpool.tile([128, HEAD_DIM], F32, tag="w")
                nc.sync.dma_start(out=ws, in_=WqT[c * 128:(c + 1) * 128, :])
                for m in range(M):
                    nc.tensor.matmul(
                        psq[m],
                        ws[:, m * 128:(m + 1) * 128],
                        h999_sb[:, c, :],
                        start=(c == 0), stop=(c == C - 1))
            q999_sb = persist.tile([128, M * BPC], F32)
            for m in range(M):
                nc.vector.tensor_scalar(
                    out=q999_sb[:, m * BPC:(m + 1) * BPC],
                    in0=psq[m],
                    scalar1=bq_sb[:, m:m + 1], scalar2=None, op0=ALU.add)

            # ---- per batch ----
            for b in range(BPC):
                kTr = projpool.tile([128, M, SEQ], F32, tag="ktr")
                vTr = projpool.tile([128, M, SEQ], F32, tag="vtr")
                for n in range(NH):
                    ts = n * NT
                    hb = []
                    for c in range(C):
                        t = hpool.tile([128, NT], F32, tag="hb", name=f"hb{c}")
                        nc.sync.dma_start(
                            out=t, in_=hTr[b, c * 128:(c + 1) * 128, ts:ts + NT])
                        hb.append(t)
                    for W, bias_sb, dst in ((WkT, bk_sb, kTr), (WvT, bv_sb, vTr)):
                        psm = [ps.tile([128, NT], F32, tag="pp", name=f"psm{m}") for m in range(M)]
                        for c in range(C):
                            ws = wpool.tile([128, HEAD_DIM], F32, tag="w")
                            nc.sync.dma_start(out=ws, in_=W[c * 128:(c + 1) * 128, :])
                            for m in range(M):
                                nc.tensor.matmul(
                                    psm[m], ws[:, m * 128:(m + 1) * 128], hb[c],
                                    start=(c == 0), stop=(c == C - 1))
                        for m in range(M):
                            nc.vector.tensor_scalar(
                                out=dst[:, m, ts:ts + NT], in0=psm[m],
                                scalar1=bias_sb[:, m:m + 1], scalar2=None, op0=ALU.add)

                # s[t] = q999_b . kT[:, t] / sqrt(d)   -> [1, SEQ]
                s_sb = clspool.tile([1, SEQ], F32, tag="s")
                for n in range(NH):
                    ts = n * NT
                    pss = ps.tile([1, NT], F32, tag="pp")
                    for m in range(M):
                        col = m * BPC + b
                        nc.tensor.matmul(
                            pss, q999_sb[:, col:col + 1], kTr[:, m, ts:ts + NT],
                            start=(m == 0), stop=(m == M - 1))
                    nc.scalar.activation(
                        out=s_sb[:, ts:ts + NT], in_=pss, func=AF.Copy,
                        bias=0.0, scale=INV_SQRT_D)
                mx = clspool.tile([1, 1], F32, tag="mx")
                nc.vector.tensor_reduce(out=mx, in_=s_sb, axis=mybir.AxisListType.X,
                                        op=ALU.max)
                nc.vector.tensor_scalar(out=s_sb, in0=s_sb, scalar1=mx,
                                        scalar2=None, op0=ALU.subtract)
                # broadcast s row to 128 partitions via ones-matmul
                sbc = clspool.tile([128, SEQ], F32, tag="sbc")
                for n in range(NH):
                    ts = n * NT
                    psb = ps.tile([128, NT], F32, tag="pp")
                    nc.tensor.matmul(psb, ones_sb, s_sb[0:1, ts:ts + NT],
                                     start=True, stop=True)
                    nc.vector.tensor_copy(sbc[:, ts:ts + NT], psb)

                for m in range(M):
                    v2 = vTr[:, m, :]
                    # out999 chunk: sum_t s'[t] * vT[d, t]
                    prod = clspool.tile([128, SEQ], F32, tag="prod")
                    nc.vector.tensor_mul(prod, v2, sbc)
                    o999 = clspool.tile([128, 1], F32, tag="o999")
                    nc.vector.tensor_reduce(out=o999, in_=prod,
                                            axis=mybir.AxisListType.X, op=ALU.add)
                    # exclusive prefix min/max along reversed t
                    em = clspool.tile([128, SEQ], F32, tag="em")
                    eM = clspool.tile([128, SEQ], F32, tag="eM")
                    nc.vector.tensor_tensor_scan(
                        out=em[:, 1:SEQ], data0=v2[:, 0:SEQ - 1], data1=v2[:, 0:SEQ - 1],
                        initial=3.0e38, op0=ALU.min, op1=ALU.bypass)
                    nc.vector.tensor_tensor_scan(
                        out=eM[:, 1:SEQ], data0=v2[:, 0:SEQ - 1], data1=v2[:, 0:SEQ - 1],
                        initial=-3.0e38, op0=ALU.max, op1=ALU.bypass)
                    cls = clspool.tile([128, SEQ], F32, tag="cls")
                    msk = clspool.tile([128, SEQ], U32, tag="msk")
                    nc.vector.tensor_copy(cls, nan_c.to_broadcast([128, SEQ]))
                    nc.vector.tensor_scalar(out=msk, in0=em, scalar1=0.0,
                                            scalar2=None, op0=ALU.is_gt)
                    nc.vector.copy_predicated(cls, msk, ninf_c.to_broadcast([128, SEQ]))
                    nc.vector.tensor_scalar(out=msk, in0=eM, scalar1=0.0,
                                            scalar2=None, op0=ALU.is_lt)
                    nc.vector.copy_predicated(cls, msk, pinf_c.to_broadcast([128, SEQ]))
                    # reversed col 0 == original row 999 (finite attention row)
                    nc.vector.tensor_copy(cls[:, 0:1], o999)
                    nc.sync.dma_start(out=out[b, m * 128:(m + 1) * 128, :], in_=cls)

    split_multi_waits(nc)
    return nc


_NC = None


def _get_program():
    global _NC
    if _NC is None:
        _NC = build_program()
    return _NC


def kernel(**inputs) -> np.ndarray:
    h = np.asarray(inputs["h"], dtype=np.float32)
    Wq = np.asarray(inputs["Wq"], dtype=np.float32)
    Wk = np.asarray(inputs["Wk"], dtype=np.float32)
    Wv = np.asarray(inputs["Wv"], dtype=np.float32)
    bq = np.asarray(inputs["bq"], dtype=np.float32)
    bk = np.asarray(inputs["bk"], dtype=np.float32)
    bv = np.asarray(inputs["bv"], dtype=np.float32)

    nc = _get_program()

    # host-side shard prep: reversed-t transposed activations, transposed weights
    hTr = np.ascontiguousarray(h[:, ::-1, :].transpose(0, 2, 1))   # [B, HIDDEN, SEQ]
    h999T = np.ascontiguousarray(h[:, SEQ - 1, :].T)               # [HIDDEN, B]
    WqTs = [np.ascontiguousarray(Wq[hd].T) for hd in range(NUM_HEAD)]
    WkTs = [np.ascontiguousarray(Wk[hd].T) for hd in range(NUM_HEAD)]
    WvTs = [np.ascontiguousarray(Wv[hd].T) for hd in range(NUM_HEAD)]

    in_maps = []
    for core in range(NCORES):
        hd, g = divmod(core, 2)
        sl = slice(g * BPC, (g + 1) * BPC)
        in_maps.append({
            "hTr": hTr[sl],
            "h999T": np.ascontiguousarray(h999T[:, sl]),
            "WqT": WqTs[hd], "WkT": WkTs[hd], "WvT": WvTs[hd],
            "bq": np.ascontiguousarray(bq[hd]),
            "bk": np.ascontiguousarray(bk[hd]),
            "bv": np.ascontiguousarray(bv[hd]),
        })

    res = run_bass_kernel_spmd(nc, in_maps, list(range(NCORES)))

    outp = np.empty((BATCH, SEQ, NUM_HEAD * HEAD_DIM), dtype=np.float32)
    for core in range(NCORES):
        hd, g = divmod(core, 2)
        blk = res.results[core]["out"]            # [BPC, HEAD_DIM, SEQ] reversed t
        outp[g * BPC:(g + 1) * BPC, :, hd * HEAD_DIM:(hd + 1) * HEAD_DIM] = (
            blk.transpose(0, 2, 1)[:, ::-1, :])
    return outp


# revision 5
# speedup vs baseline: 50.8694x; 50.8694x over previous
"""Trainium2 Bass kernel for nn_Attention_46110768890297.

Math: reference computes per-head q,k,v projections, causal-masked scores
with -inf, max-subtraction (NO softmax), then s @ v.  Because masked
positions carry -inf into the final matmul, output rows l < SEQ-1 are
exactly determined by IEEE rules from the SIGNS of v over the suffix
m > l:  all v>0 -> -inf, all v<0 -> +inf, else NaN.  Only row SEQ-1 is
finite and needs the actual attention computation.

Device computation per core (core = (head, batch-group of 5)):
  kT, vT = (h @ WkT + bk).T, (h @ WvT + bv).T   in [d, t] layout (t reversed)
  q999   = h[:, 999] @ WqT + bq
  s      = (q999 . kT) / sqrt(d); s -= max(s); out999 = sum_t s[t] * vT[:, t]
  rows 0..998: exclusive prefix min/max scans of vT along reversed t give
  the all-pos / all-neg suffix classification -> write -inf/+inf/NaN.
Host reassembles the 8 per-core [5, 1024, 1000] blocks.
"""

import numpy as np

import concourse.bass as bass
from concourse import mybir
from concourse.tile import TileContext
from concourse.bass_utils import run_bass_kernel_spmd

BATCH, SEQ, HIDDEN = 10, 1000, 4096
HEAD_DIM, NUM_HEAD = 1024, 4
NCORES = 8
BPC = BATCH // 2          # batches per core (2 batch groups)
C = HIDDEN // 128         # 32 contraction chunks
M = HEAD_DIM // 128       # 8 output-dim chunks
NT = 500                  # t-tile (PSUM free dim limit 512 fp32)
NH = SEQ // NT            # 2 t halves
F32 = mybir.dt.float32
F32R = mybir.dt.float32r
U32 = mybir.dt.uint32
USE_F32R = True  # full-rate fp32 mode for k/v projections (11-bit mantissa)
ALU = mybir.AluOpType
AF = mybir.ActivationFunctionType
INV_SQRT_D = 1.0 / float(np.sqrt(np.float32(HEAD_DIM)))


def split_multi_waits(nc):
    """This walrus build allows only ONE sync-wait per instruction struct.
    Move extra waits onto standalone EventSemaphore instructions placed
    immediately before, on the same engine (sequencers are in-order)."""
    k = 0
    for f in nc.m.functions:
        for bb in f.blocks:
            out, changed = [], False
            for inst in bb.instructions:
                si = inst.sync_info
                waits = list(si.on_wait) if (si is not None and si.on_wait) else []
                if len(waits) > 1:
                    for w in waits[:-1]:
                        ev = mybir.InstEventSemaphore(
                            name=f"xw-{k}", engine=inst.engine, debug=inst.debug,
                            sync_info=mybir.SyncInfo(on_wait=[w], on_update=[]))
                        k += 1
                        out.append(ev)
                    inst.sync_info = mybir.SyncInfo(
                        on_wait=[waits[-1]], on_update=list(si.on_update or []))
                    changed = True
                out.append(inst)
            if changed:
                bb.instructions = out
    return nc


def build_program(reps=1):
    nc = bass.Bass()
    PDT = F32R if USE_F32R else F32
    hTr = nc.declare_dram_parameter("hTr", [BPC, HIDDEN, SEQ], PDT, isOutput=False)
    h999T = nc.declare_dram_parameter("h999T", [HIDDEN, BPC], F32, isOutput=False)
    WqT = nc.declare_dram_parameter("WqT", [HIDDEN, HEAD_DIM], F32, isOutput=False)
    WkT = nc.declare_dram_parameter("WkT", [HIDDEN, HEAD_DIM], PDT, isOutput=False)
    WvT = nc.declare_dram_parameter("WvT", [HIDDEN, HEAD_DIM], PDT, isOutput=False)
    bq = nc.declare_dram_parameter("bq", [HEAD_DIM], F32, isOutput=False)
    bk = nc.declare_dram_parameter("bk", [HEAD_DIM], F32, isOutput=False)
    bv = nc.declare_dram_parameter("bv", [HEAD_DIM], F32, isOutput=False)
    out = nc.declare_dram_parameter("out", [BPC, HEAD_DIM, SEQ], F32, isOutput=True)

    with TileContext(nc) as tc:
        with (
            tc.tile_pool(name="persist", bufs=1) as persist,
            tc.tile_pool(name="wstrip", bufs=3) as wpool,
            tc.tile_pool(name="hblk", bufs=C) as hpool,
            tc.tile_pool(name="proj", bufs=1) as projpool,
            tc.tile_pool(name="cls", bufs=2) as clspool,
            tc.tile_pool(name="ps", bufs=8, space="PSUM") as ps,
        ):
          for _rep in range(reps):
            # ---- constants / small loads ----
            h999_sb = persist.tile([128, C, BPC], F32)
            nc.sync.dma_start(out=h999_sb, in_=h999T.rearrange("(c p) b -> p c b", p=128))
            bq_sb = persist.tile([128, M], F32)
            nc.sync.dma_start(out=bq_sb, in_=bq.rearrange("(m p) -> p m", p=128))
            bk_sb = persist.tile([128, M], F32)
            nc.sync.dma_start(out=bk_sb, in_=bk.rearrange("(m p) -> p m", p=128))
            bv_sb = persist.tile([128, M], F32)
            nc.sync.dma_start(out=bv_sb, in_=bv.rearrange("(m p) -> p m", p=128))
            ones_sb = persist.tile([1, 128], F32)
            nc.vector.memset(ones_sb, 1.0)
            nan_c = persist.tile([128, 1], F32)
            nc.vector.memset(nan_c, float("nan"))
            ninf_c = persist.tile([128, 1], F32)
            nc.vector.memset(ninf_c, float("-inf"))
            pinf_c = persist.tile([128, 1], F32)
            nc.vector.memset(pinf_c, float("inf"))

            # ---- q999 for all 5 batches: q999_sb[:, m*BPC + b] ----
            # one PSUM tile per m-chunk: start=True clears has_written for the
            # whole bank, so interleaved groups must not share a bank
            psq = [ps.tile([128, BPC], F32, tag="pp", name=f"psq{m}") for m in range(M)]
            for c in range(C):
                ws = wpool.tile([128, HEAD_DIM], F32, tag="w")
                nc.sync.dma_start(out=ws, in_=WqT[c * 128:(c + 1) * 128, :])
                for m in range(M):
                    nc.tensor.matmul(
                        psq[m],
                        ws[:, m * 128:(m + 1) * 128],
                        h999_sb[:, c, :],
                        start=(c == 0), stop=(c == C - 1))
            q999_sb = persist.tile([128, M * BPC], F32)
            for m in range(M):
                nc.vector.tensor_scalar(
                    out=q999_sb[:, m * BPC:(m + 1) * BPC],
                    in0=psq[m],
                    scalar1=bq_sb[:, m:m + 1], scalar2=None, op0=ALU.add)

            # ---- per batch ----
            for b in range(BPC):
                kTr = projpool.tile([128, M, SEQ], F32, tag="ktr")
                vTr = projpool.tile([128, M, SEQ], F32, tag="vtr")
                for n in range(NH):
                    ts = n * NT
                    hb = []
                    for c in range(C):
                        t = hpool.tile([128, NT], PDT, tag="hb", name=f"hb{c}")
                        nc.sync.dma_start(
                            out=t, in_=hTr[b, c * 128:(c + 1) * 128, ts:ts + NT])
                        hb.append(t)
                    for W, bias_sb, dst in ((WkT, bk_sb, kTr), (WvT, bv_sb, vTr)):
                        psm = [ps.tile([128, NT], F32, tag="pp", name=f"psm{m}") for m in range(M)]
                        for c in range(C):
                            ws = wpool.tile([128, HEAD_DIM], PDT, tag="wkv")
                            nc.sync.dma_start(out=ws, in_=W[c * 128:(c + 1) * 128, :])
                            for m in range(M):
                                nc.tensor.matmul(
                                    psm[m], ws[:, m * 128:(m + 1) * 128], hb[c],
                                    start=(c == 0), stop=(c == C - 1))
                        for m in range(M):
                            nc.vector.tensor_scalar(
                                out=dst[:, m, ts:ts + NT], in0=psm[m],
                                scalar1=bias_sb[:, m:m + 1], scalar2=None, op0=ALU.add)

                # s[t] = q999_b . kT[:, t] / sqrt(d)   -> [1, SEQ]
                s_sb = clspool.tile([1, SEQ], F32, tag="s")
                for n in range(NH):
                    ts = n * NT
                    pss = ps.tile([1, NT], F32, tag="pp")
                    for m in range(M):
                        col = m * BPC + b
                        nc.tensor.matmul(
                            pss, q999_sb[:, col:col + 1], kTr[:, m, ts:ts + NT],
                            start=(m == 0), stop=(m == M - 1))
                    nc.scalar.activation(
                        out=s_sb[:, ts:ts + NT], in_=pss, func=AF.Copy,
                        bias=0.0, scale=INV_SQRT_D)
                mx = clspool.tile([1, 1], F32, tag="mx")
                nc.vector.tensor_reduce(out=mx, in_=s_sb, axis=mybir.AxisListType.X,
                                        op=ALU.max)
                nc.vector.tensor_scalar(out=s_sb, in0=s_sb, scalar1=mx,
                                        scalar2=None, op0=ALU.subtract)
                # broadcast s row to 128 partitions via ones-matmul
                sbc = clspool.tile([128, SEQ], F32, tag="sbc")
                for n in range(NH):
                    ts = n * NT
                    psb = ps.tile([128, NT], F32, tag="pp")
                    nc.tensor.matmul(psb, ones_sb, s_sb[0:1, ts:ts + NT],
                                     start=True, stop=True)
                    nc.vector.tensor_copy(sbc[:, ts:ts + NT], psb)

                for m in range(M):
                    v2 = vTr[:, m, :]
                    # out999 chunk: sum_t s'[t] * vT[d, t]
                    prod = clspool.tile([128, SEQ], F32, tag="prod")
                    nc.vector.tensor_mul(prod, v2, sbc)
                    o999 = clspool.tile([128, 1], F32, tag="o999")
                    nc.vector.tensor_reduce(out=o999, in_=prod,
                                            axis=mybir.AxisListType.X, op=ALU.add)
                    # exclusive prefix min/max along reversed t
                    em = clspool.tile([128, SEQ], F32, tag="em")
                    eM = clspool.tile([128, SEQ], F32, tag="eM")
                    nc.vector.tensor_tensor_scan(
                        out=em[:, 1:SEQ], data0=v2[:, 0:SEQ - 1], data1=v2[:, 0:SEQ - 1],
                        initial=3.0e38, op0=ALU.min, op1=ALU.bypass)
                    nc.vector.tensor_tensor_scan(
                        out=eM[:, 1:SEQ], data0=v2[:, 0:SEQ - 1], data1=v2[:, 0:SEQ - 1],
                        initial=-3.0e38, op0=ALU.max, op1=ALU.bypass)
                    cls = clspool.tile([128, SEQ], F32, tag="cls")
                    msk = clspool.tile([128, SEQ], U32, tag="msk")
                    nc.vector.tensor_copy(cls, nan_c.to_broadcast([128, SEQ]))
                    nc.vector.tensor_scalar(out=msk, in0=em, scalar1=0.0,
                                            scalar2=None, op0=ALU.is_gt)
                    nc.vector.copy_predicated(cls, msk, ninf_c.to_broadcast([128, SEQ]))
                    nc.vector.tensor_scalar(out=msk, in0=eM, scalar1=0.0,
                                            scalar2=None, op0=ALU.is_lt)
                    nc.vector.copy_predicated(cls, msk, pinf_c.to_broadcast([128, SEQ]))
                    # reversed col 0 == original row 999 (finite attention row)
                    nc.vector.tensor_copy(cls[:, 0:1], o999)
                    nc.sync.dma_start(out=out[b, m * 128:(m + 1) * 128, :], in_=cls)

    split_multi_waits(nc)
    return nc


_NC = {}


def _get_program(reps=1):
    if reps not in _NC:
        _NC[reps] = build_program(reps)
    return _NC[reps]


def prepare_in_maps(inputs):
    h = np.asarray(inputs["h"], dtype=np.float32)
    Wq = np.asarray(inputs["Wq"], dtype=np.float32)
    Wk = np.asarray(inputs["Wk"], dtype=np.float32)
    Wv = np.asarray(inputs["Wv"], dtype=np.float32)
    bq = np.asarray(inputs["bq"], dtype=np.float32)
    bk = np.asarray(inputs["bk"], dtype=np.float32)
    bv = np.asarray(inputs["bv"], dtype=np.float32)
    hTr = _round_f32r(np.ascontiguousarray(h[:, ::-1, :].transpose(0, 2, 1)))
    h999T = np.ascontiguousarray(h[:, SEQ - 1, :].T)
    WqTs = [np.ascontiguousarray(Wq[hd].T) for hd in range(NUM_HEAD)]
    WkTs = [_round_f32r(np.ascontiguousarray(Wk[hd].T)) for hd in range(NUM_HEAD)]
    WvTs = [_round_f32r(np.ascontiguousarray(Wv[hd].T)) for hd in range(NUM_HEAD)]
    in_maps = []
    for core in range(NCORES):
        hd, g = divmod(core, 2)
        sl = slice(g * BPC, (g + 1) * BPC)
        in_maps.append({
            "hTr": hTr[sl],
            "h999T": np.ascontiguousarray(h999T[:, sl]),
            "WqT": WqTs[hd], "WkT": WkTs[hd], "WvT": WvTs[hd],
            "bq": np.ascontiguousarray(bq[hd]),
            "bk": np.ascontiguousarray(bk[hd]),
            "bv": np.ascontiguousarray(bv[hd]),
        })
    return in_maps


def _round_f32r(a):
    """Round fp32 array to 11-bit mantissa (the f32r hi part), RNE-ish."""
    if not USE_F32R:
        return a
    u = a.view(np.uint32) if a.flags['C_CONTIGUOUS'] else np.ascontiguousarray(a).view(np.uint32)
    u2 = (u.astype(np.uint64) + 0x800) & 0xFFFFF000
    return u2.astype(np.uint32).view(np.float32)


def kernel(**inputs) -> np.ndarray:
    nc = _get_program()
    in_maps = prepare_in_maps(inputs)
    res = run_bass_kernel_spmd(nc, in_maps, list(range(NCORES)))

    outp = np.empty((BATCH, SEQ, NUM_HEAD * HEAD_DIM), dtype=np.float32)
    for core in range(NCORES):
        hd, g = divmod(core, 2)
        blk = res.results[core]["out"]            # [BPC, HEAD_DIM, SEQ] reversed t
        outp[g * BPC:(g + 1) * BPC, :, hd * HEAD_DIM:(hd + 1) * HEAD_DIM] = (
            blk.transpose(0, 2, 1)[:, ::-1, :])
    return outp
